# revision 59
# baseline (speedup 1.0000x reference)
"""Spatial self-attention (SAGAN-style) kernel for 8 Trainium2 NeuronCores.

Math (per batch b):
    xf  = x[b].reshape(C, N)                       # C=256, N=4096
    qT  = wq @ xf + bq                             # [32, N]
    kT  = wk @ xf + bk                             # [32, N]
    V   = wv @ xf + bv                             # [C, N]
    E^T = kT.T @ qT                                # [m, n]  (keys on partitions)
    A'  = exp(E^T)          (no max-subtraction: |E| < 29, safe in fp32)
    s   = colsum(A')                               # softmax denominator per query
    out = gamma * (V @ A / s) + x

Sharding: core i handles batch b = i//2, query half h = i%2 (2048 queries).
Each core computes kT / V^T for the full 4096 keys of its batch. The host
rotates xf per-core so the core's 2048 query columns always sit at columns
0..2047 (attention reductions are permutation-invariant over keys), which
keeps the SPMD program uniform with no separate query tensor.

Device layout choices (O^T formulation):
  - E^T orientation (keys on PSUM partitions) so A' feeds the output matmul
    as the moving operand with zero transposes anywhere.
  - V^T [m, c] computed directly on PE (lhsT = xf block, rhs = (g*wv).T).
  - softmax denominator via an all-ones [128,128] stationary matmul that
    accumulates alongside the two output-channel matmuls -> s replicated
    across all 128 partitions for free.
  - gamma folded into wv/bv on the host; residual added on DVE (x^T loaded
    bf16, late); output stored bf16.
  - all matmuls in bf16 with fp32 PSUM accumulation (bf16 LDWEIGHTS gets
    fast-weight-load and hides behind the matmul). fp8 DoubleRow was tried
    for the output matmuls and measured SLOWER than bf16 here (FD=257).
  - wq/wk are column-stacked 4x on the host so the projection matmuls write
    the 4x-row-replicated qT/kT (for the row-packed E quads) directly - no
    SBUF->SBUF replication DMAs. qT/kT extraction: q on DVE, k on ACT
    (Identity with per-partition bias AP).
  - phase 1 interleaves the vT projection pairs between q/k chunks, psv
    tiles cycle the 4 psT slots (idle until phase 2) for a 4-deep PSUM
    pipeline, and vT extraction is split DVE (fused bias) / ACT Copy +
    Pool in-place bias add (Pool cannot read PSUM).
  - phase 2 lags the out-matmul emission LAG groups behind the E+exp
    emission so exp(g) runs on ACT while PE does out(g-LAG) - removes the
    E->exp->out refill bubble at group/chunk boundaries. ot tiles allocate
    lazily inside emit_out so the psT generation fence lands after the
    previous chunk's lagged writes (allocating earlier deadlocks).
  - few, big DMAs on the sync engine only (issue ~0.6us each; gpsimd DMAs
    go through the slow software-DGE path - avoid). Weights/biases ship as
    two packed blobs.
  - E PSUM tiles are [128,1024] pairs so one ACT instruction exponentiates
    two banks (amortizes the per-op ACT overhead).
  - walrus allows at most ONE semaphore wait per TPB instruction; Tile's
    surplus waits are legalized post-hoc (_strip_self_waits drops redundant
    same-engine waits, _split_multi_waits moves the rest onto single-wait
    Drains inserted before the offender).
Measured on HW: 139.9us (prev session baseline) -> 117.8us, rel_l2 2.4e-3.
"""

import ml_dtypes
import numpy as np

import concourse.bass as bass
import concourse.mybir as mybir
import concourse.tile as tile
from concourse.bass import ts
from concourse.bass_utils import run_bass_kernel_spmd

B, C, HH, WW = 4, 256, 64, 64
N = HH * WW          # 4096 spatial positions
D = 32               # C // 8 head dim
NCORES = 8
NQ = N * B // NCORES  # 2048 queries per core
MB = N // 128        # 32 key blocks
QCH = NQ // 512      # 4 query chunks per core
KCH = N // 512       # 8 chunks across keys

F32 = mybir.dt.float32
F32R = mybir.dt.float32r
BF16 = mybir.dt.bfloat16
AF = mybir.ActivationFunctionType
OP = mybir.AluOpType


VW = C + 2          # vT block width: 256 channels + ones col + pad


def _build():
    nc = bass.Bass()
    xfb = nc.declare_dram_parameter("xfb", [C, N], BF16, isOutput=False)
    xtq = nc.declare_dram_parameter("xtq", [NQ, C], BF16, isOutput=False)
    # bf16 weights in one blob, wq/wk pre-stacked 4x along columns so the
    # projection matmuls write the 4x-row-replicated qT/kT directly:
    # [wq4_0|wq4_1|wk4_0|wk4_1|wv_0|wv_1] = [128, 1024]
    wblob = nc.declare_dram_parameter("wblob", [128, 4 * C], BF16,
                                      isOutput=False)
    # all f32 biases in one blob: bq|bk|bv2 = [128, 514]
    bblob = nc.declare_dram_parameter("bblob", [128, 2 + 2 * C], F32,
                                      isOutput=False)
    outT = nc.declare_dram_parameter("outT", [NQ, C], BF16, isOutput=True)
    NSL = NQ // 128     # 16 query slices of 128

    with tile.TileContext(nc) as tc:
        with (
            tc.tile_pool(name="const", bufs=1) as constp,
            tc.tile_pool(name="xfp", bufs=1) as xfp,
            tc.tile_pool(name="big", bufs=1) as bigp,
            tc.tile_pool(name="apool", bufs=6) as apool,
            tc.tile_pool(name="fin", bufs=2) as finp,
            tc.tile_pool(name="psT", bufs=1, space="PSUM") as psT,
            tc.tile_pool(name="psE", bufs=2, space="PSUM") as psE,
        ):
            xf_b = [xfp.tile([128, N], BF16, name=f"xfb{i}") for i in range(2)]
            xt = xfp.tile([128, NSL * C], BF16, name="xt")
            w_t = constp.tile([128, 4 * C], BF16, name="wb")
            wq_t = lambda i: w_t[:, i * 128:(i + 1) * 128]
            wk_t = lambda i: w_t[:, 256 + i * 128:256 + (i + 1) * 128]
            wv_t = lambda i: w_t[:, 512 + i * C:512 + (i + 1) * C]
            b_t = constp.tile([128, 2 + 2 * C], F32, name="bb")
            # kT/qT: rows 0:32 computed, replicated for 4x row-packed E
            kT = bigp.tile([128, N], BF16, name="kT")
            qT = bigp.tile([128, NQ], BF16, name="qT")
            # vT block m at cols [m*VW, m*VW+C); col m*VW+C holds 1.0 so the
            # output matmul also accumulates the softmax denominator
            vT = bigp.tile([128, MB * VW], BF16, name="vT")

            # DMA issue is ~0.6us per dma_start on the sync engine; few, big
            # transfers (each fans out across the 16 HW DMA engines). Order:
            # everything the first q/k chunk needs goes first.
            nc.sync.dma_start(w_t[:], wblob[:, :])
            nc.sync.dma_start(b_t[:], bblob[:, :])
            HQ = 512   # first slab: exactly what q/k chunk 0 + vT pairs 0-1 need
            for i in range(2):
                nc.sync.dma_start(xf_b[i][:, 0:HQ], xfb[i * 128:(i + 1) * 128, 0:HQ])
            for i in range(2):
                nc.sync.dma_start(xf_b[i][:, HQ:N], xfb[i * 128:(i + 1) * 128, HQ:N])
            # ones column of every vT block
            nc.vector.memset(vT[:, C::VW], 1.0)

            # phase 1: qT, kT, vT projections. vT pairs interleave between
            # q/k chunks so PE always has matmul work while DVE/ACT drain the
            # extractions (q on DVE, k on ACT-Identity, vT split DVE/ACT+Pool)
            def vt_pair(mp):
                m0, m1 = 2 * mp, 2 * mp + 1
                # psv tiles cycle the 4 psT slots (idle until phase 2) for a
                # 4-deep PSUM pipeline; psE keeps the q/k and E tiles
                psv0 = psT.tile([128, C], F32, tag=f"ot{(2 * mp) % 4}",
                                name=f"psv{m0}")
                psv1 = psT.tile([128, C], F32, tag=f"ot{(2 * mp + 1) % 4}",
                                name=f"psv{m1}")
                for cb in range(2):
                    nc.tensor.matmul(
                        psv0[:], lhsT=xf_b[cb][:, ts(m0, 128)], rhs=wv_t(cb),
                        start=(cb == 0), stop=(cb == 1), skip_group_check=True)
                    nc.tensor.matmul(
                        psv1[:], lhsT=xf_b[cb][:, ts(m1, 128)], rhs=wv_t(cb),
                        start=(cb == 0), stop=(cb == 1), skip_group_check=True)
                # DVE does m0 with fused bias; the (phase-1-idle) ACT engine
                # copies m1 PSUM->SBUF and Pool adds the bias in place (SBUF
                # only - no PSUM access on Pool)
                nc.vector.tensor_tensor(vT[:, m0 * VW:m0 * VW + C], psv0[:],
                                        b_t[:, 2:2 + C], OP.add)
                nc.scalar.activation(vT[:, m1 * VW:m1 * VW + C], psv1[:],
                                     AF.Copy)
                nc.gpsimd.tensor_tensor(vT[:, m1 * VW:m1 * VW + C],
                                        vT[:, m1 * VW:m1 * VW + C],
                                        b_t[:, 2:2 + C], OP.add)

            for ch in range(QCH):
                psq = psE.tile([128, 512], F32, tag="e", name=f"psq{ch}")
                psk = psE.tile([128, 512], F32, tag="e", name=f"psk{ch}")
                for cb in range(2):
                    nc.tensor.matmul(
                        psq[:], lhsT=wq_t(cb), rhs=xf_b[cb][:, ts(ch, 512)],
                        start=(cb == 0), stop=(cb == 1), skip_group_check=True)
                    nc.tensor.matmul(
                        psk[:], lhsT=wk_t(cb), rhs=xf_b[cb][:, ts(ch, 512)],
                        start=(cb == 0), stop=(cb == 1), skip_group_check=True)
                nc.vector.tensor_scalar_add(qT[:, ts(ch, 512)], psq[:],
                                            b_t[:, 0:1])
                nc.scalar.activation(kT[:, ts(ch, 512)], psk[:],
                                     AF.Identity, bias=b_t[:, 1:2])
                vt_pair(2 * ch)
                vt_pair(2 * ch + 1)
            for ch in range(QCH, KCH):
                ps = psE.tile([128, 512], F32, tag="e", name=f"psk{ch}")
                ps2 = psE.tile([128, 512], F32, tag="e", name=f"psk2_{ch}")
                h = 256
                for cb in range(2):
                    nc.tensor.matmul(
                        ps[:, 0:h], lhsT=wk_t(cb),
                        rhs=xf_b[cb][:, ch * 512:ch * 512 + h],
                        start=(cb == 0), stop=(cb == 1), skip_group_check=True)
                    nc.tensor.matmul(
                        ps2[:, 0:h], lhsT=wk_t(cb),
                        rhs=xf_b[cb][:, ch * 512 + h:(ch + 1) * 512],
                        start=(cb == 0), stop=(cb == 1), skip_group_check=True)
                nc.vector.tensor_scalar_add(kT[:, ch * 512:ch * 512 + h],
                                            ps[:, 0:h], b_t[:, 1:2])
                nc.scalar.activation(kT[:, ch * 512 + h:(ch + 1) * 512],
                                     ps2[:, 0:h], AF.Identity,
                                     bias=b_t[:, 1:2])
                vt_pair(2 * ch)
                vt_pair(2 * ch + 1)
            # x^T residual slices, first needed at chunk-0 finalize (~55us):
            # issued late so the transfer doesn't compete with the xfb slab
            nc.sync.dma_start(
                xt[:].rearrange("p (s c) -> p s c", c=C),
                xtq.rearrange("(s p) c -> p s c", p=128))

            # phase 2: E^T -> exp -> O^T = A'.T @ [gV^T | 1], one 512-query
            # chunk at a time; 4 query-slice accumulators of [128, C+1].
            # The out-matmul emission lags the E+exp emission by one group so
            # the exp of group g runs on ACT while PE does group g-1's out
            # matmuls - this removes the E->exp->out refill bubble at group
            # and chunk boundaries.
            ot_of = {}

            def emit_out(ch, g, ats):
                if g == 0:
                    # allocate here, not at chunk start: with the one-group
                    # lag the previous chunk's last out-writes are emitted
                    # after the chunk-start point, and the slot generation
                    # fence must come after them
                    ot_of[ch] = [psT.tile([128, C + 1], F32, tag=f"ot{j}",
                                          name=f"ot{j}_{ch}") for j in range(4)]
                ot = ot_of[ch]
                for i in range(4):
                    m = 4 * g + i
                    st, sp = (m == 0), (m == MB - 1)
                    asl = ats[i // 2]
                    base = 512 * (i % 2)
                    for j in range(4):
                        nc.tensor.matmul(
                            ot[j][:],
                            lhsT=asl[:, base + 128 * j:base + 128 * (j + 1)],
                            rhs=vT[:, m * VW:m * VW + C + 1],
                            start=st, stop=sp, skip_group_check=True)

            def emit_fin(ch, act_assist=False):
                ot = ot_of[ch]
                for j in range(4):
                    sl = 4 * ch + j
                    r = finp.tile([128, 1], F32, tag="r", bufs=8,
                                  name=f"r{ch}_{j}")
                    nc.vector.reciprocal(r[:], ot[j][:, C:C + 1])
                    t = finp.tile([128, C], F32, tag="t", bufs=4,
                                  name=f"t{ch}_{j}")
                    if act_assist and j % 2 == 1:
                        # last chunk only: ACT is idle in the tail, so let it
                        # do half the ot*(1/s) scaling (Copy with per-query
                        # scale) while DVE handles the other slices
                        nc.scalar.activation(t[:], ot[j][:, 0:C], AF.Copy,
                                             scale=r[:, 0:1])
                    else:
                        nc.vector.tensor_scalar_mul(t[:], ot[j][:, 0:C],
                                                    r[:, 0:1])
                    f = finp.tile([128, C], BF16, tag="f", bufs=4,
                                  name=f"f{ch}_{j}")
                    nc.vector.tensor_tensor(f[:], t[:], xt[:, ts(sl, C)], OP.add)
                    nc.sync.dma_start(outT[sl * 128:(sl + 1) * 128, :], f[:])

            pend = []
            LAG = 2   # out-emission groups behind E+exp emission
            for ch in range(QCH):
                for g in range(MB // 4):
                    ats = []
                    for p in range(2):      # [128,1024] pair: 2 key blocks
                        e = psE.tile([128, 1024], F32, tag="e",
                                     name=f"e{ch}_{g}_{p}")
                        for i in range(2):
                            row = 2 * p + i
                            nc.tensor.matmul(
                                e[:, 512 * i:512 * (i + 1)],
                                lhsT=kT[32 * row:32 * (row + 1),
                                        ts(4 * g + row, 128)],
                                rhs=qT[32 * row:32 * (row + 1), ts(ch, 512)],
                                start=True, stop=True, skip_group_check=True,
                                tile_position=(32 * row, 0),
                            )
                        a = apool.tile([128, 1024], BF16, tag="a",
                                       name=f"a{ch}_{g}_{p}")
                        nc.scalar.activation(a[:], e[:], AF.Exp)
                        ats.append(a)
                    pend.append((ch, g, ats))
                    if len(pend) > LAG:
                        pch, pg, pats = pend.pop(0)
                        emit_out(pch, pg, pats)
                        if pg == MB // 4 - 1:
                            emit_fin(pch)
            for pch, pg, pats in pend:
                emit_out(pch, pg, pats)
                if pg == MB // 4 - 1:
                    emit_fin(pch, act_assist=True)
    _strip_self_waits(nc)
    _split_multi_waits(nc)
    return nc


_ENGINE_SEM_PREFIX = {
    "EngineType.PE": "PE_",
    "EngineType.DVE": "DVE_",
    "EngineType.Activation": "Activation_",
    "EngineType.Pool": "Pool_",
    "EngineType.SP": "SP_",
}


def _strip_self_waits(nc):
    """Drop same-engine semaphore waits from multi-wait TPB instructions.

    Walrus allows exactly one sync wait per TPB instruction. Tile emits
    redundant self-engine waits (WAW on pool-slot reuse, RAW from same-engine
    producers): each engine executes its queue in order, so a wait on the
    engine's own semaphore is always satisfied by program order. Dropping
    them collapses every instruction to at most one (cross-engine) wait.
    """
    for bb in nc.m.functions[0].blocks:
        for inst in bb.instructions:
            si = inst.sync_info
            if si is None:
                continue
            w = si.on_wait
            if len(w) <= 1 or inst.opcode == "Drain":
                continue
            pfx = _ENGINE_SEM_PREFIX.get(str(inst.engine))
            if pfx is None:
                continue
            kept = [x for x in w if not x.ant_name.startswith(pfx)]
            if kept and len(kept) < len(w):
                si.on_wait = kept


def _split_multi_waits(nc):
    """Walrus allows one sync wait per TPB instruction; move surplus waits
    onto dedicated single-wait Drain instructions inserted just before the
    offender (same engine, executes in order)."""
    import bass_rust
    cnt = 0
    for bb in nc.m.functions[0].blocks:
        il = bb.instructions
        i = 0
        while i < len(il):
            inst = il[i]
            si = inst.sync_info
            w = si.on_wait if si else []
            if len(w) > 1:
                for j, wait in enumerate(w[:-1]):
                    d = mybir.InstDrain(name=f"{inst.name}-w{j}", ins=[], outs=[],
                                        bass_is_fusable=False)
                    d.engine = inst.engine
                    d.sync_info = bass_rust.SyncInfo(on_wait=[wait], on_update=[])
                    il.insert(i, d)
                    i += 1
                    cnt += 1
                si.on_wait = [w[-1]]
            i += 1
    return cnt


def audit_matmul_waits(nc):
    """Max sync-wait count on any Matmult (walrus limit: 1)."""
    worst = (0, None)
    for bb in nc.m.functions[0].blocks:
        for inst in bb.instructions:
            if inst.opcode != "Matmult":
                continue
            w = inst.sync_info.on_wait if inst.sync_info else []
            if len(w) > worst[0]:
                worst = (len(w), (inst.name, [x.ant_name for x in w]))
    return worst


_NC_CACHE = None


def _get_nc():
    global _NC_CACHE
    if _NC_CACHE is None:
        _NC_CACHE = _build()
    return _NC_CACHE


def kernel(x, wq, bq, wk, bk, wv, bv, gamma, _trace=False):
    f32 = lambda a: np.ascontiguousarray(np.asarray(a, dtype=np.float32))
    bf16 = lambda a: np.ascontiguousarray(np.asarray(a, dtype=np.float32)
                                          .astype(ml_dtypes.bfloat16))
    x = f32(x)
    g = float(np.asarray(gamma).reshape(-1)[0])
    xfull = x.reshape(B, C, N)
    wqT = np.asarray(wq, np.float32).T    # [C, D]
    wkT = np.asarray(wk, np.float32).T
    wvT = (g * np.asarray(wv, np.float32)).T
    # bf16 weight blob [128, 1024]: wq4_0|wq4_1|wk4_0|wk4_1|wv0|wv1 with
    # wq/wk column-stacked 4x (projection then emits replicated qT/kT rows)
    wb = np.concatenate([np.tile(wqT[0:128], (1, 4)),
                         np.tile(wqT[128:256], (1, 4)),
                         np.tile(wkT[0:128], (1, 4)),
                         np.tile(wkT[128:256], (1, 4)),
                         wvT[0:128], wvT[128:256]], axis=1)
    # f32 bias blob [128, 514]: bq|bk|bv doubled
    bq4 = np.tile(np.asarray(bq, np.float32).reshape(D, 1), (128 // D, 1))
    bk4 = np.tile(np.asarray(bk, np.float32).reshape(D, 1), (128 // D, 1))
    bv2 = np.tile((g * np.asarray(bv, np.float32)).reshape(1, C), (128, 2))
    shared = {
        "wblob": bf16(wb),
        "bblob": f32(np.concatenate([bq4, bk4, bv2], axis=1)),
    }
    in_maps = []
    for core in range(NCORES):
        b, h = core // 2, core % 2
        m = dict(shared)
        if h == 0:
            xr = xfull[b]
        else:
            # rotate so this core's query half sits at columns 0..NQ-1;
            # key order is irrelevant (attention reduces over all keys)
            xr = np.concatenate([xfull[b][:, NQ:], xfull[b][:, :NQ]], axis=1)
        m["xfb"] = bf16(xr)
        m["xtq"] = bf16(xr[:, :NQ].T)
        in_maps.append(m)

    res = run_bass_kernel_spmd(_get_nc(), in_maps, list(range(NCORES)),
                               trace=_trace)
    full = np.empty((B, C, N), np.float32)
    for core in range(NCORES):
        b, h = core // 2, core % 2
        full[b][:, h * NQ:(h + 1) * NQ] = (
            res.results[core]["outT"].astype(np.float32).T)
    out = full.reshape(B, C, HH, WW)
    if _trace:
        return out, res
    return out



# revision 60
# speedup vs baseline: 1.0171x; 1.0171x over previous
"""Spatial self-attention (SAGAN-style) kernel for 8 Trainium2 NeuronCores.

Math (per batch b):
    xf  = x[b].reshape(C, N)                       # C=256, N=4096
    qT  = wq @ xf + bq                             # [32, N]
    kT  = wk @ xf + bk                             # [32, N]
    V   = wv @ xf + bv                             # [C, N]
    E^T = kT.T @ qT                                # [m, n]  (keys on partitions)
    A'  = exp(E^T)          (no max-subtraction: |E| < 29, safe in fp32)
    s   = colsum(A')                               # softmax denominator per query
    out = gamma * (V @ A / s) + x

Sharding: core i handles batch b = i//2, query half h = i%2 (2048 queries).
Each core computes kT / V^T for the full 4096 keys of its batch. The host
rotates xf per-core so the core's 2048 query columns always sit at columns
0..2047 (attention reductions are permutation-invariant over keys), which
keeps the SPMD program uniform with no separate query tensor.

Device layout choices (O^T formulation):
  - E^T orientation (keys on PSUM partitions) so A' feeds the output matmul
    as the moving operand with zero transposes anywhere.
  - V^T [m, c] computed directly on PE (lhsT = xf block, rhs = (g*wv).T).
  - softmax denominator via an all-ones [128,128] stationary matmul that
    accumulates alongside the two output-channel matmuls -> s replicated
    across all 128 partitions for free.
  - gamma folded into wv/bv on the host; residual added on DVE (x^T loaded
    bf16, late); output stored bf16.
  - all matmuls in bf16 with fp32 PSUM accumulation (bf16 LDWEIGHTS gets
    fast-weight-load and hides behind the matmul). fp8 DoubleRow was tried
    for the output matmuls and measured SLOWER than bf16 here (FD=257).
  - wq/wk are column-stacked 4x on the host so the projection matmuls write
    the 4x-row-replicated qT/kT (for the row-packed E quads) directly - no
    SBUF->SBUF replication DMAs. qT/kT extraction: q on DVE, k on ACT
    (Identity with per-partition bias AP).
  - phase 1 interleaves the vT projection pairs between q/k chunks, psv
    tiles cycle the 4 psT slots (idle until phase 2) for a 4-deep PSUM
    pipeline, and vT extraction is split DVE (fused bias) / ACT Copy +
    Pool in-place bias add (Pool cannot read PSUM).
  - phase 2 lags the out-matmul emission LAG groups behind the E+exp
    emission so exp(g) runs on ACT while PE does out(g-LAG) - removes the
    E->exp->out refill bubble at group/chunk boundaries. ot tiles allocate
    lazily inside emit_out so the psT generation fence lands after the
    previous chunk's lagged writes (allocating earlier deadlocks).
  - few, big DMAs on the sync engine only (issue ~0.6us each; gpsimd DMAs
    go through the slow software-DGE path - avoid). Weights/biases ship as
    two packed blobs.
  - E PSUM tiles are [128,1024] pairs so one ACT instruction exponentiates
    two banks (amortizes the per-op ACT overhead).
  - walrus allows at most ONE semaphore wait per TPB instruction; Tile's
    surplus waits are legalized post-hoc (_strip_self_waits drops redundant
    same-engine waits, _split_multi_waits moves the rest onto single-wait
    Drains inserted before the offender).
Measured on HW: 139.9us (prev session baseline) -> 117.8us, rel_l2 2.4e-3.
"""

import ml_dtypes
import numpy as np

import concourse.bass as bass
import concourse.mybir as mybir
import concourse.tile as tile
from concourse.bass import ts
from concourse.bass_utils import run_bass_kernel_spmd

B, C, HH, WW = 4, 256, 64, 64
N = HH * WW          # 4096 spatial positions
D = 32               # C // 8 head dim
NCORES = 8
NQ = N * B // NCORES  # 2048 queries per core
MB = N // 128        # 32 key blocks
QCH = NQ // 512      # 4 query chunks per core
KCH = N // 512       # 8 chunks across keys

F32 = mybir.dt.float32
F32R = mybir.dt.float32r
BF16 = mybir.dt.bfloat16
AF = mybir.ActivationFunctionType
OP = mybir.AluOpType


VW = C + 2          # vT block width: 256 channels + ones col + pad


def _build():
    nc = bass.Bass()
    xfb = nc.declare_dram_parameter("xfb", [C, N], BF16, isOutput=False)
    xtq = nc.declare_dram_parameter("xtq", [NQ, C], BF16, isOutput=False)
    # bf16 weights in one blob, wq/wk pre-stacked 4x along columns so the
    # projection matmuls write the 4x-row-replicated qT/kT directly:
    # [wq4_0|wq4_1|wk4_0|wk4_1|wv_0|wv_1] = [128, 1024]
    wblob = nc.declare_dram_parameter("wblob", [128, 4 * C], BF16,
                                      isOutput=False)
    # all f32 biases in one blob: bq|bk|bv2 = [128, 514]
    bblob = nc.declare_dram_parameter("bblob", [128, 2 + 2 * C], F32,
                                      isOutput=False)
    outT = nc.declare_dram_parameter("outT", [NQ, C], BF16, isOutput=True)
    NSL = NQ // 128     # 16 query slices of 128

    with tile.TileContext(nc) as tc:
        with (
            tc.tile_pool(name="const", bufs=1) as constp,
            tc.tile_pool(name="xfp", bufs=1) as xfp,
            tc.tile_pool(name="big", bufs=1) as bigp,
            tc.tile_pool(name="apool", bufs=6) as apool,
            tc.tile_pool(name="fin", bufs=2) as finp,
            tc.tile_pool(name="psT", bufs=1, space="PSUM") as psT,
            tc.tile_pool(name="psE", bufs=2, space="PSUM") as psE,
        ):
            xf_b = [xfp.tile([128, N], BF16, name=f"xfb{i}") for i in range(2)]
            xt = xfp.tile([128, NSL * C], BF16, name="xt")
            w_t = constp.tile([128, 4 * C], BF16, name="wb")
            wq_t = lambda i: w_t[:, i * 128:(i + 1) * 128]
            wk_t = lambda i: w_t[:, 256 + i * 128:256 + (i + 1) * 128]
            wv_t = lambda i: w_t[:, 512 + i * C:512 + (i + 1) * C]
            b_t = constp.tile([128, 2 + 2 * C], F32, name="bb")
            # kT/qT: rows 0:32 computed, replicated for 4x row-packed E
            kT = bigp.tile([128, N], BF16, name="kT")
            qT = bigp.tile([128, NQ], BF16, name="qT")
            # vT block m at cols [m*VW, m*VW+C); col m*VW+C holds 1.0 so the
            # output matmul also accumulates the softmax denominator
            vT = bigp.tile([128, MB * VW], BF16, name="vT")

            # DMA issue is ~0.6us per dma_start on the sync engine; few, big
            # transfers (each fans out across the 16 HW DMA engines). Order:
            # everything the first q/k chunk needs goes first.
            nc.sync.dma_start(w_t[:], wblob[:, :])
            nc.sync.dma_start(b_t[:], bblob[:, :])
            HQ = 512   # first slab: exactly what q/k chunk 0 + vT pairs 0-1 need
            for i in range(2):
                nc.sync.dma_start(xf_b[i][:, 0:HQ], xfb[i * 128:(i + 1) * 128, 0:HQ])
            for i in range(2):
                nc.sync.dma_start(xf_b[i][:, HQ:N], xfb[i * 128:(i + 1) * 128, HQ:N])
            # ones column of every vT block
            nc.vector.memset(vT[:, C::VW], 1.0)

            # phase 1: qT, kT, vT projections. vT pairs interleave between
            # q/k chunks so PE always has matmul work while DVE/ACT drain the
            # extractions (q on DVE, k on ACT-Identity, vT split DVE/ACT+Pool)
            def vt_pair(mp):
                m0, m1 = 2 * mp, 2 * mp + 1
                # psv tiles cycle the 4 psT slots (idle until phase 2) for a
                # 4-deep PSUM pipeline; psE keeps the q/k and E tiles
                psv0 = psT.tile([128, C], F32, tag=f"ot{(2 * mp) % 4}",
                                name=f"psv{m0}")
                psv1 = psT.tile([128, C], F32, tag=f"ot{(2 * mp + 1) % 4}",
                                name=f"psv{m1}")
                for cb in range(2):
                    nc.tensor.matmul(
                        psv0[:], lhsT=xf_b[cb][:, ts(m0, 128)], rhs=wv_t(cb),
                        start=(cb == 0), stop=(cb == 1), skip_group_check=True)
                    nc.tensor.matmul(
                        psv1[:], lhsT=xf_b[cb][:, ts(m1, 128)], rhs=wv_t(cb),
                        start=(cb == 0), stop=(cb == 1), skip_group_check=True)
                # DVE does m0 with fused bias; the (phase-1-idle) ACT engine
                # copies m1 PSUM->SBUF and Pool adds the bias in place (SBUF
                # only - no PSUM access on Pool)
                nc.vector.tensor_tensor(vT[:, m0 * VW:m0 * VW + C], psv0[:],
                                        b_t[:, 2:2 + C], OP.add)
                nc.scalar.activation(vT[:, m1 * VW:m1 * VW + C], psv1[:],
                                     AF.Copy)
                nc.gpsimd.tensor_tensor(vT[:, m1 * VW:m1 * VW + C],
                                        vT[:, m1 * VW:m1 * VW + C],
                                        b_t[:, 2:2 + C], OP.add)

            for ch in range(QCH):
                psq = psE.tile([128, 512], F32, tag="e", name=f"psq{ch}")
                psk = psE.tile([128, 512], F32, tag="e", name=f"psk{ch}")
                for cb in range(2):
                    nc.tensor.matmul(
                        psq[:], lhsT=wq_t(cb), rhs=xf_b[cb][:, ts(ch, 512)],
                        start=(cb == 0), stop=(cb == 1), skip_group_check=True)
                    nc.tensor.matmul(
                        psk[:], lhsT=wk_t(cb), rhs=xf_b[cb][:, ts(ch, 512)],
                        start=(cb == 0), stop=(cb == 1), skip_group_check=True)
                nc.vector.tensor_scalar_add(qT[:, ts(ch, 512)], psq[:],
                                            b_t[:, 0:1])
                nc.scalar.activation(kT[:, ts(ch, 512)], psk[:],
                                     AF.Identity, bias=b_t[:, 1:2])
                vt_pair(2 * ch)
                vt_pair(2 * ch + 1)
            for ch in range(QCH, KCH):
                ps = psE.tile([128, 512], F32, tag="e", name=f"psk{ch}")
                ps2 = psE.tile([128, 512], F32, tag="e", name=f"psk2_{ch}")
                h = 256
                for cb in range(2):
                    nc.tensor.matmul(
                        ps[:, 0:h], lhsT=wk_t(cb),
                        rhs=xf_b[cb][:, ch * 512:ch * 512 + h],
                        start=(cb == 0), stop=(cb == 1), skip_group_check=True)
                    nc.tensor.matmul(
                        ps2[:, 0:h], lhsT=wk_t(cb),
                        rhs=xf_b[cb][:, ch * 512 + h:(ch + 1) * 512],
                        start=(cb == 0), stop=(cb == 1), skip_group_check=True)
                nc.vector.tensor_scalar_add(kT[:, ch * 512:ch * 512 + h],
                                            ps[:, 0:h], b_t[:, 1:2])
                nc.scalar.activation(kT[:, ch * 512 + h:(ch + 1) * 512],
                                     ps2[:, 0:h], AF.Identity,
                                     bias=b_t[:, 1:2])
                vt_pair(2 * ch)
                vt_pair(2 * ch + 1)
            # x^T residual slices, first needed at chunk-0 finalize (~55us):
            # issued late so the transfer doesn't compete with the xfb slab
            nc.sync.dma_start(
                xt[:].rearrange("p (s c) -> p s c", c=C),
                xtq.rearrange("(s p) c -> p s c", p=128))

            # phase 2: E^T -> exp -> O^T = A'.T @ [gV^T | 1], one 512-query
            # chunk at a time; 4 query-slice accumulators of [128, C+1].
            # The out-matmul emission lags the E+exp emission by one group so
            # the exp of group g runs on ACT while PE does group g-1's out
            # matmuls - this removes the E->exp->out refill bubble at group
            # and chunk boundaries.
            ot_of = {}

            def emit_out(ch, g, ats):
                if g == 0:
                    # allocate here, not at chunk start: with the one-group
                    # lag the previous chunk's last out-writes are emitted
                    # after the chunk-start point, and the slot generation
                    # fence must come after them
                    ot_of[ch] = [psT.tile([128, C + 1], F32, tag=f"ot{j}",
                                          name=f"ot{j}_{ch}") for j in range(4)]
                ot = ot_of[ch]
                last = (g == MB // 4 - 1)
                # final group runs j-outer so ot[0] stops 12 matmuls earlier
                # and the per-slice finalization pipeline starts sooner
                order = ([(i, j) for j in range(4) for i in range(4)] if last
                         else [(i, j) for i in range(4) for j in range(4)])
                for i, j in order:
                    m = 4 * g + i
                    st, sp = (m == 0), (m == MB - 1)
                    asl = ats[i // 2]
                    base = 512 * (i % 2)
                    nc.tensor.matmul(
                        ot[j][:],
                        lhsT=asl[:, base + 128 * j:base + 128 * (j + 1)],
                        rhs=vT[:, m * VW:m * VW + C + 1],
                        start=st, stop=sp, skip_group_check=True)

            def emit_fin(ch, act_assist=False):
                ot = ot_of[ch]
                for j in range(4):
                    sl = 4 * ch + j
                    r = finp.tile([128, 1], F32, tag="r", bufs=8,
                                  name=f"r{ch}_{j}")
                    nc.vector.reciprocal(r[:], ot[j][:, C:C + 1])
                    t = finp.tile([128, C], F32, tag="t", bufs=4,
                                  name=f"t{ch}_{j}")
                    if act_assist and j % 2 == 1:
                        # last chunk only: ACT is idle in the tail, so let it
                        # do half the ot*(1/s) scaling (Copy with per-query
                        # scale) while DVE handles the other slices
                        nc.scalar.activation(t[:], ot[j][:, 0:C], AF.Copy,
                                             scale=r[:, 0:1])
                    else:
                        nc.vector.tensor_scalar_mul(t[:], ot[j][:, 0:C],
                                                    r[:, 0:1])
                    f = finp.tile([128, C], BF16, tag="f", bufs=4,
                                  name=f"f{ch}_{j}")
                    nc.vector.tensor_tensor(f[:], t[:], xt[:, ts(sl, C)], OP.add)
                    nc.sync.dma_start(outT[sl * 128:(sl + 1) * 128, :], f[:])

            pend = []
            LAG = 2   # out-emission groups behind E+exp emission
            for ch in range(QCH):
                for g in range(MB // 4):
                    ats = []
                    for p in range(2):      # [128,1024] pair: 2 key blocks
                        e = psE.tile([128, 1024], F32, tag="e",
                                     name=f"e{ch}_{g}_{p}")
                        for i in range(2):
                            row = 2 * p + i
                            nc.tensor.matmul(
                                e[:, 512 * i:512 * (i + 1)],
                                lhsT=kT[32 * row:32 * (row + 1),
                                        ts(4 * g + row, 128)],
                                rhs=qT[32 * row:32 * (row + 1), ts(ch, 512)],
                                start=True, stop=True, skip_group_check=True,
                                tile_position=(32 * row, 0),
                            )
                        a = apool.tile([128, 1024], BF16, tag="a",
                                       name=f"a{ch}_{g}_{p}")
                        nc.scalar.activation(a[:], e[:], AF.Exp)
                        ats.append(a)
                    pend.append((ch, g, ats))
                    if len(pend) > LAG:
                        pch, pg, pats = pend.pop(0)
                        emit_out(pch, pg, pats)
                        if pg == MB // 4 - 1:
                            emit_fin(pch)
            for pch, pg, pats in pend:
                emit_out(pch, pg, pats)
                if pg == MB // 4 - 1:
                    emit_fin(pch, act_assist=True)
    _strip_self_waits(nc)
    _split_multi_waits(nc)
    return nc


_ENGINE_SEM_PREFIX = {
    "EngineType.PE": "PE_",
    "EngineType.DVE": "DVE_",
    "EngineType.Activation": "Activation_",
    "EngineType.Pool": "Pool_",
    "EngineType.SP": "SP_",
}


def _strip_self_waits(nc):
    """Drop same-engine semaphore waits from multi-wait TPB instructions.

    Walrus allows exactly one sync wait per TPB instruction. Tile emits
    redundant self-engine waits (WAW on pool-slot reuse, RAW from same-engine
    producers): each engine executes its queue in order, so a wait on the
    engine's own semaphore is always satisfied by program order. Dropping
    them collapses every instruction to at most one (cross-engine) wait.
    """
    for bb in nc.m.functions[0].blocks:
        for inst in bb.instructions:
            si = inst.sync_info
            if si is None:
                continue
            w = si.on_wait
            if len(w) <= 1 or inst.opcode == "Drain":
                continue
            pfx = _ENGINE_SEM_PREFIX.get(str(inst.engine))
            if pfx is None:
                continue
            kept = [x for x in w if not x.ant_name.startswith(pfx)]
            if kept and len(kept) < len(w):
                si.on_wait = kept


def _split_multi_waits(nc):
    """Walrus allows one sync wait per TPB instruction; move surplus waits
    onto dedicated single-wait Drain instructions inserted just before the
    offender (same engine, executes in order)."""
    import bass_rust
    cnt = 0
    for bb in nc.m.functions[0].blocks:
        il = bb.instructions
        i = 0
        while i < len(il):
            inst = il[i]
            si = inst.sync_info
            w = si.on_wait if si else []
            if len(w) > 1:
                for j, wait in enumerate(w[:-1]):
                    d = mybir.InstDrain(name=f"{inst.name}-w{j}", ins=[], outs=[],
                                        bass_is_fusable=False)
                    d.engine = inst.engine
                    d.sync_info = bass_rust.SyncInfo(on_wait=[wait], on_update=[])
                    il.insert(i, d)
                    i += 1
                    cnt += 1
                si.on_wait = [w[-1]]
            i += 1
    return cnt


def audit_matmul_waits(nc):
    """Max sync-wait count on any Matmult (walrus limit: 1)."""
    worst = (0, None)
    for bb in nc.m.functions[0].blocks:
        for inst in bb.instructions:
            if inst.opcode != "Matmult":
                continue
            w = inst.sync_info.on_wait if inst.sync_info else []
            if len(w) > worst[0]:
                worst = (len(w), (inst.name, [x.ant_name for x in w]))
    return worst


_NC_CACHE = None


def _get_nc():
    global _NC_CACHE
    if _NC_CACHE is None:
        _NC_CACHE = _build()
    return _NC_CACHE


def kernel(x, wq, bq, wk, bk, wv, bv, gamma, _trace=False):
    f32 = lambda a: np.ascontiguousarray(np.asarray(a, dtype=np.float32))
    bf16 = lambda a: np.ascontiguousarray(np.asarray(a, dtype=np.float32)
                                          .astype(ml_dtypes.bfloat16))
    x = f32(x)
    g = float(np.asarray(gamma).reshape(-1)[0])
    xfull = x.reshape(B, C, N)
    wqT = np.asarray(wq, np.float32).T    # [C, D]
    wkT = np.asarray(wk, np.float32).T
    wvT = (g * np.asarray(wv, np.float32)).T
    # bf16 weight blob [128, 1024]: wq4_0|wq4_1|wk4_0|wk4_1|wv0|wv1 with
    # wq/wk column-stacked 4x (projection then emits replicated qT/kT rows)
    wb = np.concatenate([np.tile(wqT[0:128], (1, 4)),
                         np.tile(wqT[128:256], (1, 4)),
                         np.tile(wkT[0:128], (1, 4)),
                         np.tile(wkT[128:256], (1, 4)),
                         wvT[0:128], wvT[128:256]], axis=1)
    # f32 bias blob [128, 514]: bq|bk|bv doubled
    bq4 = np.tile(np.asarray(bq, np.float32).reshape(D, 1), (128 // D, 1))
    bk4 = np.tile(np.asarray(bk, np.float32).reshape(D, 1), (128 // D, 1))
    bv2 = np.tile((g * np.asarray(bv, np.float32)).reshape(1, C), (128, 2))
    shared = {
        "wblob": bf16(wb),
        "bblob": f32(np.concatenate([bq4, bk4, bv2], axis=1)),
    }
    in_maps = []
    for core in range(NCORES):
        b, h = core // 2, core % 2
        m = dict(shared)
        if h == 0:
            xr = xfull[b]
        else:
            # rotate so this core's query half sits at columns 0..NQ-1;
            # key order is irrelevant (attention reduces over all keys)
            xr = np.concatenate([xfull[b][:, NQ:], xfull[b][:, :NQ]], axis=1)
        m["xfb"] = bf16(xr)
        m["xtq"] = bf16(xr[:, :NQ].T)
        in_maps.append(m)

    res = run_bass_kernel_spmd(_get_nc(), in_maps, list(range(NCORES)),
                               trace=_trace)
    full = np.empty((B, C, N), np.float32)
    for core in range(NCORES):
        b, h = core // 2, core % 2
        full[b][:, h * NQ:(h + 1) * NQ] = (
            res.results[core]["outT"].astype(np.float32).T)
    out = full.reshape(B, C, HH, WW)
    if _trace:
        return out, res
    return out



# revision 61
# speedup vs baseline: 1.0417x; 1.0242x over previous
"""Spatial self-attention (SAGAN-style) kernel for 8 Trainium2 NeuronCores.

Math (per batch b):
    xf  = x[b].reshape(C, N)                       # C=256, N=4096
    qT  = wq @ xf + bq                             # [32, N]
    kT  = wk @ xf + bk                             # [32, N]
    V   = wv @ xf + bv                             # [C, N]
    E^T = kT.T @ qT                                # [m, n]  (keys on partitions)
    A'  = exp(E^T)          (no max-subtraction: |E| < 29, safe in fp32)
    s   = colsum(A')                               # softmax denominator per query
    out = gamma * (V @ A / s) + x

Sharding: core i handles batch b = i//2, query half h = i%2 (2048 queries).
Each core computes kT / V^T for the full 4096 keys of its batch. The host
rotates xf per-core so the core's 2048 query columns always sit at columns
0..2047 (attention reductions are permutation-invariant over keys), which
keeps the SPMD program uniform with no separate query tensor.

Device layout choices (O^T formulation):
  - E^T orientation (keys on PSUM partitions) so A' feeds the output matmul
    as the moving operand with zero transposes anywhere.
  - V^T [m, c] computed directly on PE (lhsT = xf block, rhs = (g*wv).T).
  - softmax denominator via an all-ones [128,128] stationary matmul that
    accumulates alongside the two output-channel matmuls -> s replicated
    across all 128 partitions for free.
  - gamma folded into wv/bv on the host; residual added on DVE (x^T loaded
    bf16, late); output stored bf16.
  - all matmuls in bf16 with fp32 PSUM accumulation (bf16 LDWEIGHTS gets
    fast-weight-load and hides behind the matmul). fp8 DoubleRow was tried
    for the output matmuls and measured SLOWER than bf16 here (FD=257).
  - wq/wk are column-stacked 4x on the host so the projection matmuls write
    the 4x-row-replicated qT/kT (for the row-packed E quads) directly - no
    SBUF->SBUF replication DMAs. qT/kT extraction: q on DVE, k on ACT
    (Identity with per-partition bias AP).
  - phase 1 interleaves the vT projection pairs between q/k chunks, psv
    tiles cycle the 4 psT slots (idle until phase 2) for a 4-deep PSUM
    pipeline, and vT extraction is split DVE (fused bias) / ACT Copy +
    Pool in-place bias add (Pool cannot read PSUM).
  - phase 2 lags the out-matmul emission LAG groups behind the E+exp
    emission so exp(g) runs on ACT while PE does out(g-LAG) - removes the
    E->exp->out refill bubble at group/chunk boundaries. ot tiles allocate
    lazily inside emit_out so the psT generation fence lands after the
    previous chunk's lagged writes (allocating earlier deadlocks).
  - few, big DMAs on the sync engine only (issue ~0.6us each; gpsimd DMAs
    go through the slow software-DGE path - avoid). Weights/biases ship as
    two packed blobs.
  - E PSUM tiles are [128,1024] pairs so one ACT instruction exponentiates
    two banks (amortizes the per-op ACT overhead).
  - walrus allows at most ONE semaphore wait per TPB instruction; Tile's
    surplus waits are legalized post-hoc (_strip_self_waits drops redundant
    same-engine waits, _split_multi_waits moves the rest onto single-wait
    Drains inserted before the offender).
Measured on HW: 139.9us (prev session baseline) -> 117.8us, rel_l2 2.4e-3.
"""

import ml_dtypes
import numpy as np

import concourse.bass as bass
import concourse.mybir as mybir
import concourse.tile as tile
from concourse.bass import ts
from concourse.bass_utils import run_bass_kernel_spmd

B, C, HH, WW = 4, 256, 64, 64
N = HH * WW          # 4096 spatial positions
D = 32               # C // 8 head dim
NCORES = 8
NQ = N * B // NCORES  # 2048 queries per core
MB = N // 128        # 32 key blocks
QCH = NQ // 512      # 4 query chunks per core
KCH = N // 512       # 8 chunks across keys

F32 = mybir.dt.float32
F32R = mybir.dt.float32r
BF16 = mybir.dt.bfloat16
AF = mybir.ActivationFunctionType
OP = mybir.AluOpType


VW = C + 2          # vT block width: 256 channels + ones col + pad


def _build():
    nc = bass.Bass()
    xfb = nc.declare_dram_parameter("xfb", [C, N], BF16, isOutput=False)
    # xtq pre-packed on the host to the device xt layout [128, NSL*C]
    # (slice-major per partition) so the load is one contiguous transfer
    # instead of ~2k small gather packets
    xtq = nc.declare_dram_parameter("xtq", [128, (NQ // 128) * C], BF16,
                                    isOutput=False)
    # bf16 weights in one blob, wq/wk pre-stacked 4x along columns so the
    # projection matmuls write the 4x-row-replicated qT/kT directly:
    # [wq4_0|wq4_1|wk4_0|wk4_1|wv_0|wv_1] = [128, 1024]
    wqkb = nc.declare_dram_parameter("wqkb", [128, 2 * C], BF16,
                                     isOutput=False)
    wvb = nc.declare_dram_parameter("wvb", [128, 2 * C], BF16,
                                    isOutput=False)
    # all f32 biases in one blob: bq|bk|bv2 = [128, 514]
    bblob = nc.declare_dram_parameter("bblob", [128, 2 + 2 * C], F32,
                                      isOutput=False)
    outT = nc.declare_dram_parameter("outT", [NQ, C], BF16, isOutput=True)
    NSL = NQ // 128     # 16 query slices of 128

    with tile.TileContext(nc) as tc:
        with (
            tc.tile_pool(name="const", bufs=1) as constp,
            tc.tile_pool(name="xfp", bufs=1) as xfp,
            tc.tile_pool(name="big", bufs=1) as bigp,
            tc.tile_pool(name="apool", bufs=6) as apool,
            tc.tile_pool(name="fin", bufs=2) as finp,
            tc.tile_pool(name="psT", bufs=1, space="PSUM") as psT,
            tc.tile_pool(name="psE", bufs=2, space="PSUM") as psE,
        ):
            xf_b = [xfp.tile([128, N], BF16, name=f"xfb{i}") for i in range(2)]
            xt = xfp.tile([128, NSL * C], BF16, name="xt")
            wqk_t = constp.tile([128, 2 * C], BF16, name="wqkb")
            wv_tt = constp.tile([128, 2 * C], BF16, name="wvb")
            wq_t = lambda i: wqk_t[:, i * 128:(i + 1) * 128]
            wk_t = lambda i: wqk_t[:, 256 + i * 128:256 + (i + 1) * 128]
            wv_t = lambda i: wv_tt[:, i * C:(i + 1) * C]
            b_t = constp.tile([128, 2 + 2 * C], F32, name="bb")
            # kT/qT: rows 0:32 computed, replicated for 4x row-packed E
            kT = bigp.tile([128, N], BF16, name="kT")
            qT = bigp.tile([128, NQ], BF16, name="qT")
            # vT block m at cols [m*VW, m*VW+C); col m*VW+C holds 1.0 so the
            # output matmul also accumulates the softmax denominator
            vT = bigp.tile([128, MB * VW], BF16, name="vT")

            # DMA issue is ~0.6us per dma_start on the sync engine; few, big
            # transfers (each fans out across the 16 HW DMA engines). Order:
            # everything the first q/k chunk needs goes first.
            nc.sync.dma_start(wqk_t[:], wqkb[:, :])
            nc.sync.dma_start(b_t[:], bblob[:, :])
            nc.sync.dma_start(wv_tt[:], wvb[:, :])
            HQ = 512   # first slab: exactly what q/k chunk 0 + vT pairs 0-1 need
            for i in range(2):
                nc.sync.dma_start(xf_b[i][:, 0:HQ], xfb[i * 128:(i + 1) * 128, 0:HQ])
            for i in range(2):
                nc.sync.dma_start(xf_b[i][:, HQ:N], xfb[i * 128:(i + 1) * 128, HQ:N])
            # ones column of every vT block
            nc.vector.memset(vT[:, C::VW], 1.0)

            # phase 1: qT, kT, vT projections. vT pairs interleave between
            # q/k chunks so PE always has matmul work while DVE/ACT drain the
            # extractions (q on DVE, k on ACT-Identity, vT split DVE/ACT+Pool)
            def vt_pair(mp):
                m0, m1 = 2 * mp, 2 * mp + 1
                # psv tiles cycle the 4 psT slots (idle until phase 2) for a
                # 4-deep PSUM pipeline; psE keeps the q/k and E tiles
                psv0 = psT.tile([128, C], F32, tag=f"ot{(2 * mp) % 4}",
                                name=f"psv{m0}")
                psv1 = psT.tile([128, C], F32, tag=f"ot{(2 * mp + 1) % 4}",
                                name=f"psv{m1}")
                for cb in range(2):
                    nc.tensor.matmul(
                        psv0[:], lhsT=xf_b[cb][:, ts(m0, 128)], rhs=wv_t(cb),
                        start=(cb == 0), stop=(cb == 1), skip_group_check=True)
                    nc.tensor.matmul(
                        psv1[:], lhsT=xf_b[cb][:, ts(m1, 128)], rhs=wv_t(cb),
                        start=(cb == 0), stop=(cb == 1), skip_group_check=True)
                # DVE does m0 with fused bias; the (phase-1-idle) ACT engine
                # copies m1 PSUM->SBUF and Pool adds the bias in place (SBUF
                # only - no PSUM access on Pool)
                nc.vector.tensor_tensor(vT[:, m0 * VW:m0 * VW + C], psv0[:],
                                        b_t[:, 2:2 + C], OP.add)
                nc.scalar.activation(vT[:, m1 * VW:m1 * VW + C], psv1[:],
                                     AF.Copy)
                nc.gpsimd.tensor_tensor(vT[:, m1 * VW:m1 * VW + C],
                                        vT[:, m1 * VW:m1 * VW + C],
                                        b_t[:, 2:2 + C], OP.add)

            for ch in range(QCH):
                psq = psE.tile([128, 512], F32, tag="e", name=f"psq{ch}")
                psk = psE.tile([128, 512], F32, tag="e", name=f"psk{ch}")
                for cb in range(2):
                    nc.tensor.matmul(
                        psq[:], lhsT=wq_t(cb), rhs=xf_b[cb][:, ts(ch, 512)],
                        start=(cb == 0), stop=(cb == 1), skip_group_check=True)
                    nc.tensor.matmul(
                        psk[:], lhsT=wk_t(cb), rhs=xf_b[cb][:, ts(ch, 512)],
                        start=(cb == 0), stop=(cb == 1), skip_group_check=True)
                nc.vector.tensor_scalar_add(qT[:, ts(ch, 512)], psq[:],
                                            b_t[:, 0:1])
                nc.scalar.activation(kT[:, ts(ch, 512)], psk[:],
                                     AF.Identity, bias=b_t[:, 1:2])
                vt_pair(2 * ch)
                vt_pair(2 * ch + 1)
            for ch in range(QCH, KCH):
                ps = psE.tile([128, 512], F32, tag="e", name=f"psk{ch}")
                ps2 = psE.tile([128, 512], F32, tag="e", name=f"psk2_{ch}")
                h = 256
                for cb in range(2):
                    nc.tensor.matmul(
                        ps[:, 0:h], lhsT=wk_t(cb),
                        rhs=xf_b[cb][:, ch * 512:ch * 512 + h],
                        start=(cb == 0), stop=(cb == 1), skip_group_check=True)
                    nc.tensor.matmul(
                        ps2[:, 0:h], lhsT=wk_t(cb),
                        rhs=xf_b[cb][:, ch * 512 + h:(ch + 1) * 512],
                        start=(cb == 0), stop=(cb == 1), skip_group_check=True)
                nc.vector.tensor_scalar_add(kT[:, ch * 512:ch * 512 + h],
                                            ps[:, 0:h], b_t[:, 1:2])
                nc.scalar.activation(kT[:, ch * 512 + h:(ch + 1) * 512],
                                     ps2[:, 0:h], AF.Identity,
                                     bias=b_t[:, 1:2])
                vt_pair(2 * ch)
                vt_pair(2 * ch + 1)
            # x^T residual slices, first needed at chunk-0 finalize (~55us):
            # issued late so the transfer doesn't compete with the xfb slab
            nc.sync.dma_start(xt[:], xtq[:, :])

            # phase 2: E^T -> exp -> O^T = A'.T @ [gV^T | 1], one 512-query
            # chunk at a time; 4 query-slice accumulators of [128, C+1].
            # The out-matmul emission lags the E+exp emission by one group so
            # the exp of group g runs on ACT while PE does group g-1's out
            # matmuls - this removes the E->exp->out refill bubble at group
            # and chunk boundaries.
            ot_of = {}

            def emit_out(ch, g, ats):
                if g == 0:
                    # allocate here, not at chunk start: with the one-group
                    # lag the previous chunk's last out-writes are emitted
                    # after the chunk-start point, and the slot generation
                    # fence must come after them
                    ot_of[ch] = [psT.tile([128, C + 1], F32, tag=f"ot{j}",
                                          name=f"ot{j}_{ch}") for j in range(4)]
                ot = ot_of[ch]
                last = (g == MB // 4 - 1)
                # final group runs j-outer so ot[0] stops 12 matmuls earlier
                # and the per-slice finalization pipeline starts sooner
                order = ([(i, j) for j in range(4) for i in range(4)] if last
                         else [(i, j) for i in range(4) for j in range(4)])
                for i, j in order:
                    m = 4 * g + i
                    st, sp = (m == 0), (m == MB - 1)
                    asl = ats[i // 2]
                    base = 512 * (i % 2)
                    nc.tensor.matmul(
                        ot[j][:],
                        lhsT=asl[:, base + 128 * j:base + 128 * (j + 1)],
                        rhs=vT[:, m * VW:m * VW + C + 1],
                        start=st, stop=sp, skip_group_check=True)

            def emit_fin(ch, act_assist=False):
                ot = ot_of[ch]
                for j in range(4):
                    sl = 4 * ch + j
                    r = finp.tile([128, 1], F32, tag="r", bufs=8,
                                  name=f"r{ch}_{j}")
                    nc.vector.reciprocal(r[:], ot[j][:, C:C + 1])
                    t = finp.tile([128, C], F32, tag="t", bufs=4,
                                  name=f"t{ch}_{j}")
                    if act_assist and j % 2 == 1:
                        # last chunk only: ACT is idle in the tail, so let it
                        # do half the ot*(1/s) scaling (Copy with per-query
                        # scale) while DVE handles the other slices
                        nc.scalar.activation(t[:], ot[j][:, 0:C], AF.Copy,
                                             scale=r[:, 0:1])
                    else:
                        nc.vector.tensor_scalar_mul(t[:], ot[j][:, 0:C],
                                                    r[:, 0:1])
                    f = finp.tile([128, C], BF16, tag="f", bufs=4,
                                  name=f"f{ch}_{j}")
                    nc.vector.tensor_tensor(f[:], t[:], xt[:, ts(sl, C)], OP.add)
                    seng = nc.scalar if act_assist and j % 2 == 1 else nc.sync
                    seng.dma_start(outT[sl * 128:(sl + 1) * 128, :], f[:])

            pend = []
            LAG = 2   # out-emission groups behind E+exp emission
            for ch in range(QCH):
                for g in range(MB // 4):
                    ats = []
                    for p in range(2):      # [128,1024] pair: 2 key blocks
                        e = psE.tile([128, 1024], F32, tag="e",
                                     name=f"e{ch}_{g}_{p}")
                        for i in range(2):
                            row = 2 * p + i
                            nc.tensor.matmul(
                                e[:, 512 * i:512 * (i + 1)],
                                lhsT=kT[32 * row:32 * (row + 1),
                                        ts(4 * g + row, 128)],
                                rhs=qT[32 * row:32 * (row + 1), ts(ch, 512)],
                                start=True, stop=True, skip_group_check=True,
                                tile_position=(32 * row, 0),
                            )
                        a = apool.tile([128, 1024], BF16, tag="a",
                                       name=f"a{ch}_{g}_{p}")
                        nc.scalar.activation(a[:], e[:], AF.Exp)
                        ats.append(a)
                    pend.append((ch, g, ats))
                    if len(pend) > LAG:
                        pch, pg, pats = pend.pop(0)
                        emit_out(pch, pg, pats)
                        if pg == MB // 4 - 1:
                            emit_fin(pch)
            for pch, pg, pats in pend:
                emit_out(pch, pg, pats)
                if pg == MB // 4 - 1:
                    emit_fin(pch, act_assist=True)
    _strip_self_waits(nc)
    _split_multi_waits(nc)
    return nc


_ENGINE_SEM_PREFIX = {
    "EngineType.PE": "PE_",
    "EngineType.DVE": "DVE_",
    "EngineType.Activation": "Activation_",
    "EngineType.Pool": "Pool_",
    "EngineType.SP": "SP_",
}


def _strip_self_waits(nc):
    """Drop same-engine semaphore waits from multi-wait TPB instructions.

    Walrus allows exactly one sync wait per TPB instruction. Tile emits
    redundant self-engine waits (WAW on pool-slot reuse, RAW from same-engine
    producers): each engine executes its queue in order, so a wait on the
    engine's own semaphore is always satisfied by program order. Dropping
    them collapses every instruction to at most one (cross-engine) wait.
    """
    for bb in nc.m.functions[0].blocks:
        for inst in bb.instructions:
            si = inst.sync_info
            if si is None:
                continue
            w = si.on_wait
            if len(w) <= 1 or inst.opcode == "Drain":
                continue
            pfx = _ENGINE_SEM_PREFIX.get(str(inst.engine))
            if pfx is None:
                continue
            kept = [x for x in w if not x.ant_name.startswith(pfx)]
            if kept and len(kept) < len(w):
                si.on_wait = kept


def _split_multi_waits(nc):
    """Walrus allows one sync wait per TPB instruction; move surplus waits
    onto dedicated single-wait Drain instructions inserted just before the
    offender (same engine, executes in order)."""
    import bass_rust
    cnt = 0
    for bb in nc.m.functions[0].blocks:
        il = bb.instructions
        i = 0
        while i < len(il):
            inst = il[i]
            si = inst.sync_info
            w = si.on_wait if si else []
            if len(w) > 1:
                for j, wait in enumerate(w[:-1]):
                    d = mybir.InstDrain(name=f"{inst.name}-w{j}", ins=[], outs=[],
                                        bass_is_fusable=False)
                    d.engine = inst.engine
                    d.sync_info = bass_rust.SyncInfo(on_wait=[wait], on_update=[])
                    il.insert(i, d)
                    i += 1
                    cnt += 1
                si.on_wait = [w[-1]]
            i += 1
    return cnt


def audit_matmul_waits(nc):
    """Max sync-wait count on any Matmult (walrus limit: 1)."""
    worst = (0, None)
    for bb in nc.m.functions[0].blocks:
        for inst in bb.instructions:
            if inst.opcode != "Matmult":
                continue
            w = inst.sync_info.on_wait if inst.sync_info else []
            if len(w) > worst[0]:
                worst = (len(w), (inst.name, [x.ant_name for x in w]))
    return worst


_NC_CACHE = None


def _get_nc():
    global _NC_CACHE
    if _NC_CACHE is None:
        _NC_CACHE = _build()
    return _NC_CACHE


def kernel(x, wq, bq, wk, bk, wv, bv, gamma, _trace=False):
    f32 = lambda a: np.ascontiguousarray(np.asarray(a, dtype=np.float32))
    bf16 = lambda a: np.ascontiguousarray(np.asarray(a, dtype=np.float32)
                                          .astype(ml_dtypes.bfloat16))
    x = f32(x)
    g = float(np.asarray(gamma).reshape(-1)[0])
    xfull = x.reshape(B, C, N)
    wqT = np.asarray(wq, np.float32).T    # [C, D]
    wkT = np.asarray(wk, np.float32).T
    wvT = (g * np.asarray(wv, np.float32)).T
    # wq/wk column-stacked 4x (projection then emits replicated qT/kT rows)
    wqk = np.concatenate([np.tile(wqT[0:128], (1, 4)),
                          np.tile(wqT[128:256], (1, 4)),
                          np.tile(wkT[0:128], (1, 4)),
                          np.tile(wkT[128:256], (1, 4))], axis=1)
    wv2 = np.concatenate([wvT[0:128], wvT[128:256]], axis=1)
    # f32 bias blob [128, 514]: bq|bk|bv doubled
    bq4 = np.tile(np.asarray(bq, np.float32).reshape(D, 1), (128 // D, 1))
    bk4 = np.tile(np.asarray(bk, np.float32).reshape(D, 1), (128 // D, 1))
    bv2 = np.tile((g * np.asarray(bv, np.float32)).reshape(1, C), (128, 2))
    shared = {
        "wqkb": bf16(wqk),
        "wvb": bf16(wv2),
        "bblob": f32(np.concatenate([bq4, bk4, bv2], axis=1)),
    }
    in_maps = []
    for core in range(NCORES):
        b, h = core // 2, core % 2
        m = dict(shared)
        if h == 0:
            xr = xfull[b]
        else:
            # rotate so this core's query half sits at columns 0..NQ-1;
            # key order is irrelevant (attention reduces over all keys)
            xr = np.concatenate([xfull[b][:, NQ:], xfull[b][:, :NQ]], axis=1)
        m["xfb"] = bf16(xr)
        xrT = xr[:, :NQ].T.reshape(NQ // 128, 128, C)
        m["xtq"] = bf16(np.ascontiguousarray(
            xrT.transpose(1, 0, 2).reshape(128, (NQ // 128) * C)))
        in_maps.append(m)

    res = run_bass_kernel_spmd(_get_nc(), in_maps, list(range(NCORES)),
                               trace=_trace)
    full = np.empty((B, C, N), np.float32)
    for core in range(NCORES):
        b, h = core // 2, core % 2
        full[b][:, h * NQ:(h + 1) * NQ] = (
            res.results[core]["outT"].astype(np.float32).T)
    out = full.reshape(B, C, HH, WW)
    if _trace:
        return out, res
    return out

